# revision 1
# baseline (speedup 1.0000x reference)
"""LocallyConnected2D (per-pixel weights, 2x2 non-overlapping patch sum, bias, relu)
for Trainium2, SPMD over 8 NeuronCores.

Math: out[b,f,or,oc] = relu( sum_{c,dr,dc} x[b,c,2or+dr,2oc+dc] * W[f,c,2or+dr,2oc+dc]
                             + bias[or,oc,f] )
with B=32, C=32, H=W=128, F=64, OR=OC=64.

Strategy:
  * Spatial-shard over OR (output rows) across 8 cores: 8 or-rows each, no halo.
  * Host-side repack (free): fold (c,dr,dc) into a single K=128 contraction axis that
    lands on the SBUF partition dim, so each output pixel is ONE K=128 matmul
    (no PSUM accumulation) and every DMA is a contiguous per-partition slab.
  * Per output pixel oc: psum[f, b] = Wk[:, f, oc].T @ xk[:, b, oc]  (lhsT=W stationary).
    Pixels are processed in parity pairs via PE column-tiling: oc even -> array cols
    0-63 (psum partitions 0-63), oc odd -> cols 64-127. Gives a [128=(parity,f), 32b]
    psum tile per pair and lets both matmuls run concurrently in the array.
  * Epilogue: single fused ScalarE activation per pair: relu(psum + bias[:,pc])
    with bias as a [128,1] per-partition vector (parity,f layout) -> SBUF out tile.
  * Output is written HBM-contiguous in a device-friendly permuted layout and
    un-permuted on the host (free).
"""

import os

import numpy as np

import concourse.bass as bass
import concourse.tile as tile
from concourse import bacc, mybir
from concourse.bass_utils import run_bass_kernel_spmd

F32 = mybir.dt.float32

B, C, H, W_ = 32, 32, 128, 128
F = 64
OR, OC = 64, 64          # full output spatial dims (stride-2, kernel-2)
NCORES = 8
ORS = OR // NCORES       # or-rows per core = 8
PC = OC // 2             # parity pairs per or-row = 32

LAST_RESULTS = None      # test harness peeks at this for exec_time_ns


NCH = 4                 # oc chunks per or-row (512 KiB W-chunk DMAs)
OCCH = OC // NCH
X_ENG = "sync"
OUT_ENG = "gpsimd"
WBUFS = 8


def _build_program():
    nc = bacc.Bacc("TRN2", target_bir_lowering=False)
    xk = nc.dram_tensor("xk", [128, ORS, OC, B], F32, kind="ExternalInput")
    wk = nc.dram_tensor("wk", [128, ORS, OC, F], F32, kind="ExternalInput")
    bk = nc.dram_tensor("bk", [128, ORS, PC], F32, kind="ExternalInput")
    out = nc.dram_tensor("out", [128, ORS, B, PC], F32, kind="ExternalOutput")

    with tile.TileContext(nc) as tc:
        with (
            tc.tile_pool(name="wp", bufs=WBUFS) as wp,
            tc.tile_pool(name="xp", bufs=WBUFS) as xp,
            tc.tile_pool(name="bp", bufs=1) as bp,
            tc.tile_pool(name="op", bufs=2) as op_,
            tc.tile_pool(name="ps", bufs=8, space=bass.MemorySpace.PSUM) as pp,
        ):
            btall = bp.tile([128, ORS, PC], F32)
            nc.sync.dma_start(out=btall[:], in_=bk[:])
            for r in range(ORS):
                bt = btall[:, r]
                ot = op_.tile([128, B, PC], F32)
                for ch in range(NCH):
                    osl = slice(ch * OCCH, (ch + 1) * OCCH)
                    wt = wp.tile([128, OCCH, F], F32)
                    nc.sync.dma_start(out=wt[:], in_=wk[:, r, osl])
                    xt = xp.tile([128, OCCH, B], F32)
                    nc.sync.dma_start(out=xt[:], in_=xk[:, r, osl])
                    for pcl in range(OCCH // 2):
                        pc = ch * (OCCH // 2) + pcl
                        ps = pp.tile([128, B], F32)
                        for par in (0, 1):
                            ol = 2 * pcl + par
                            nc.tensor.matmul(
                                ps[64 * par : 64 * par + 64, :],
                                wt[:, ol, :],      # lhsT [K=128, M=64(f)]
                                xt[:, ol, :],      # rhs  [K=128, N=32(b)]
                                start=True,
                                stop=True,
                                tile_position=(0, 64 * par),
                            )
                        nc.scalar.activation(
                            ot[:, :, pc],
                            ps[:],
                            mybir.ActivationFunctionType.Relu,
                            bias=bt[:, pc : pc + 1],
                            scale=1.0,
                        )
                nc.gpsimd.dma_start(out=out[:, r], in_=ot[:])
    nc.compile()
    return nc


_NC_CACHE = None


def kernel(x: np.ndarray, W: np.ndarray, b: np.ndarray) -> np.ndarray:
    global LAST_RESULTS, _NC_CACHE
    x = np.ascontiguousarray(x, dtype=np.float32)
    W = np.ascontiguousarray(W, dtype=np.float32)
    b = np.ascontiguousarray(b, dtype=np.float32)

    # ---- host-side repack (k = c*4 + dr*2 + dc on the partition axis) ----
    # xk_full[k, or, oc, b] = x[b, c, 2*or+dr, 2*oc+dc]
    xk_full = np.ascontiguousarray(
        x.reshape(B, C, OR, 2, OC, 2).transpose(1, 3, 5, 2, 4, 0).reshape(128, OR, OC, B)
    )
    # wk_full[k, or, oc, f] = W[f, c, 2*or+dr, 2*oc+dc]
    wk_full = np.ascontiguousarray(
        W.reshape(F, C, OR, 2, OC, 2).transpose(1, 3, 5, 2, 4, 0).reshape(128, OR, OC, F)
    )
    # reference does a RAW reshape of b (OR,OC,F)->(1,F,OR,OC): the bias used at
    # output (f,or,oc) is b viewed with raw axes (f,or,oc).
    # bk_full[parity*64+f, or, pc] = b_raw[f, or, 2*pc+parity]
    bk_full = np.ascontiguousarray(
        b.reshape(F, OR, PC, 2).transpose(3, 0, 1, 2).reshape(128, OR, PC)
    )

    if _NC_CACHE is None:
        _NC_CACHE = _build_program()
    nc = _NC_CACHE

    in_maps = []
    for i in range(NCORES):
        sl = slice(i * ORS, (i + 1) * ORS)
        in_maps.append(
            {
                "xk": np.ascontiguousarray(xk_full[:, sl]),
                "wk": np.ascontiguousarray(wk_full[:, sl]),
                "bk": np.ascontiguousarray(bk_full[:, sl]),
            }
        )

    trace = bool(os.environ.get("KERNEL_TRACE"))
    res = run_bass_kernel_spmd(nc, in_maps, core_ids=list(range(NCORES)), trace=trace)
    LAST_RESULTS = res

    # ---- host-side unpack ----
    out = np.empty((B, F, OR, OC), dtype=np.float32)
    for i in range(NCORES):
        r = res.results[i]["out"]  # [128=(parity,f), ORS, B, PC]
        blk = (
            r.reshape(2, F, ORS, B, PC)
            .transpose(3, 1, 2, 4, 0)  # -> (B, F, ORS, PC, parity)
            .reshape(B, F, ORS, OC)
        )
        out[:, :, i * ORS : (i + 1) * ORS, :] = blk
    return out



# revision 4
# speedup vs baseline: 1.6711x; 1.6711x over previous
"""LocallyConnected2D (per-pixel weights, 2x2 non-overlapping patch sum, bias, relu)
for Trainium2, SPMD over 8 NeuronCores.

Math: out[b,f,or,oc] = relu( sum_{c,dr,dc} x[b,c,2or+dr,2oc+dc] * W[f,c,2or+dr,2oc+dc]
                             + bias[or,oc,f] )
with B=32, C=32, H=W=128, F=64, OR=OC=64.

Strategy (v2, bf16):
  * Spatial-shard over OR (output rows) across 8 cores: 8 or-rows each, no halo.
  * Host-side repack (free): fold (c,dr,dc) into a single K=128 contraction axis on
    the SBUF partition dim; cast x/W to bf16 (tolerance 2e-2 >> bf16 error ~1e-3).
    Halves HBM read traffic and runs the PE at 1 cycle/row instead of fp32's 4.
  * Reads split across both HWDGE queues: W (8 MiB/core) on the SP(sync) queue,
    x (4 MiB/core) on the Act(scalar) queue, so no single queue is the bottleneck.
  * Per output pixel oc: psum[f, b] += Wk[:, f, oc].T @ xk[:, b, oc] (lhsT=W).
    Pixels processed in parity pairs via PE column-tiling (oc even -> psum
    partitions 0-63, odd -> 64-127). 16 pairs fill one PSUM bank [128, 512].
  * Bias: ONE K=16 "indicator" matmul per half-row seeds the whole PSUM bank with
    the per-(f,oc) bias (psum[p, j*32+b] = biasT[j, p]); pixel matmuls then
    accumulate onto it (start=False). Avoids 256 tiny per-pair activations.
  * Epilogue: ONE batched relu [128, 512] per half-row, alternating DVE / Pool,
    writing bf16 SBUF; output DMA'd bf16 (halves write traffic) on the Act queue.
  * Output un-permuted/upcast on the host (free).
"""

import os

import numpy as np
import ml_dtypes

import concourse.bass as bass
import concourse.tile as tile
from concourse import bacc, mybir
from concourse.bass_utils import run_bass_kernel_spmd

F32 = mybir.dt.float32
BF16 = mybir.dt.bfloat16
NP_BF16 = ml_dtypes.bfloat16

B, C, H, W_ = 32, 32, 128, 128
F = 64
OR, OC = 64, 64          # full output spatial dims (stride-2, kernel-2)
NCORES = 8
ORS = OR // NCORES       # or-rows per core = 8
PC = OC // 2             # parity pairs per or-row = 32
HPAIRS = 16              # pairs per half-row (fills one PSUM bank: 16*32=512 fp32)

LAST_RESULTS = None      # test harness peeks at this for exec_time_ns


def _build_program():
    nc = bacc.Bacc("TRN2", target_bir_lowering=False)
    xk = nc.dram_tensor("xk", [128, ORS, OC, B], BF16, kind="ExternalInput")
    wk = nc.dram_tensor("wk", [128, ORS, OC, F], BF16, kind="ExternalInput")
    bT = nc.dram_tensor("bT", [HPAIRS, ORS, 2, 128], BF16, kind="ExternalInput")
    ind = nc.dram_tensor("ind", [HPAIRS, HPAIRS * B], BF16, kind="ExternalInput")
    out = nc.dram_tensor("out", [128, ORS, 2, HPAIRS, B], BF16, kind="ExternalOutput")

    with tile.TileContext(nc) as tc:
        with (
            tc.tile_pool(name="wp", bufs=6) as wp,
            tc.tile_pool(name="xp", bufs=3) as xp,
            tc.tile_pool(name="cp", bufs=1) as cp,
            tc.tile_pool(name="op", bufs=3) as op_,
            tc.tile_pool(name="ps", bufs=8, space=bass.MemorySpace.PSUM) as pp,
        ):
            btall = cp.tile([HPAIRS, ORS, 2, 128], BF16)
            indt = cp.tile([HPAIRS, HPAIRS * B], BF16)
            nc.scalar.dma_start(out=btall[:], in_=bT[:])
            nc.scalar.dma_start(out=indt[:], in_=ind[:])

            xts = []
            for r in range(min(2, ORS)):
                xt = xp.tile([128, OC, B], BF16)
                nc.scalar.dma_start(out=xt[:], in_=xk[:, r])
                xts.append(xt)

            for r in range(ORS):
                if r + 2 < ORS:
                    xt = xp.tile([128, OC, B], BF16)
                    nc.scalar.dma_start(out=xt[:], in_=xk[:, r + 2])
                    xts.append(xt)
                xt = xts[r]
                ot = op_.tile([128, 2, HPAIRS, B], BF16)
                for h in range(2):
                    wt = wp.tile([128, 2 * HPAIRS, F], BF16)
                    nc.sync.dma_start(
                        out=wt[:], in_=wk[:, r, h * 2 * HPAIRS : (h + 1) * 2 * HPAIRS]
                    )
                    ps = pp.tile([128, HPAIRS, B], F32)
                    # Seed the whole bank with bias: psum[p, j, b] = biasT[j, p]
                    nc.tensor.matmul(
                        ps[:],
                        btall[:, r, h],            # lhsT [16, 128]
                        indt[:],                   # rhs  [16, 512]
                        start=True,
                        stop=False,
                        skip_group_check=True,
                    )
                    for j in range(HPAIRS):
                        last = j == HPAIRS - 1
                        for par in (0, 1):
                            ocl = h * 2 * HPAIRS + 2 * j + par
                            nc.tensor.matmul(
                                ps[64 * par : 64 * par + 64, j],
                                wt[:, 2 * j + par, :],   # lhsT [K=128, M=64(f)]
                                xt[:, ocl, :],           # rhs  [K=128, N=32(b)]
                                start=False,
                                stop=last and par == 1,
                                tile_position=(0, 64 * par),
                                skip_group_check=True,
                            )
                    # TRN2 GpSimd cannot access PSUM; DVE handles every relu
                    # (16 x ~0.66us, far under the DMA floor).
                    nc.vector.tensor_scalar_max(ot[:, h], ps[:], 0.0)
                nc.scalar.dma_start(out=out[:, r], in_=ot[:])
    nc.compile()
    return nc


_NC_CACHE = None


def kernel(x: np.ndarray, W: np.ndarray, b: np.ndarray) -> np.ndarray:
    global LAST_RESULTS, _NC_CACHE
    x = np.ascontiguousarray(x, dtype=np.float32)
    W = np.ascontiguousarray(W, dtype=np.float32)
    b = np.ascontiguousarray(b, dtype=np.float32)

    # ---- host-side repack (k = c*4 + dr*2 + dc on the partition axis) ----
    # xk_full[k, or, oc, b] = x[b, c, 2*or+dr, 2*oc+dc]
    xk_full = np.ascontiguousarray(
        x.reshape(B, C, OR, 2, OC, 2).transpose(1, 3, 5, 2, 4, 0).reshape(128, OR, OC, B)
    ).astype(NP_BF16)
    # wk_full[k, or, oc, f] = W[f, c, 2*or+dr, 2*oc+dc]
    wk_full = np.ascontiguousarray(
        W.reshape(F, C, OR, 2, OC, 2).transpose(1, 3, 5, 2, 4, 0).reshape(128, OR, OC, F)
    ).astype(NP_BF16)
    # reference does a RAW reshape of b (OR,OC,F)->(1,F,OR,OC): bias for output
    # (f,or,oc) is b_raw[f,or,oc] with oc = h*32 + 2*j + parity.
    # bT_full[j, or, h, parity*64+f] = b_raw[f, or, h*32+2*j+parity]
    bT_full = np.ascontiguousarray(
        b.reshape(F, OR, 2, HPAIRS, 2).transpose(3, 1, 2, 4, 0).reshape(HPAIRS, OR, 2, 128)
    ).astype(NP_BF16)
    # indicator[j, n] = 1 iff n // 32 == j  (bias-broadcast matmul rhs)
    ind_full = np.kron(np.eye(HPAIRS, dtype=np.float32), np.ones(B, np.float32)).astype(
        NP_BF16
    )

    if _NC_CACHE is None:
        _NC_CACHE = _build_program()
    nc = _NC_CACHE

    in_maps = []
    for i in range(NCORES):
        sl = slice(i * ORS, (i + 1) * ORS)
        in_maps.append(
            {
                "xk": np.ascontiguousarray(xk_full[:, sl]),
                "wk": np.ascontiguousarray(wk_full[:, sl]),
                "bT": np.ascontiguousarray(bT_full[:, sl]),
                "ind": ind_full,
            }
        )

    trace = bool(os.environ.get("KERNEL_TRACE"))
    res = run_bass_kernel_spmd(nc, in_maps, core_ids=list(range(NCORES)), trace=trace)
    LAST_RESULTS = res

    # ---- host-side unpack ----
    out = np.empty((B, F, OR, OC), dtype=np.float32)
    for i in range(NCORES):
        r = res.results[i]["out"]  # [128=(parity,f), ORS, 2(h), 16(j), 32(b)] bf16
        blk = (
            r.astype(np.float32)
            .reshape(2, F, ORS, 2, HPAIRS, B)
            .transpose(5, 1, 2, 3, 4, 0)  # -> (B, F, ORS, h, j, parity)
            .reshape(B, F, ORS, OC)
        )
        out[:, :, i * ORS : (i + 1) * ORS, :] = blk
    return out


# revision 5
# speedup vs baseline: 1.6902x; 1.0114x over previous
"""LocallyConnected2D (per-pixel weights, 2x2 non-overlapping patch sum, bias, relu)
for Trainium2, SPMD over 8 NeuronCores.

Math: out[b,f,or,oc] = relu( sum_{c,dr,dc} x[b,c,2or+dr,2oc+dc] * W[f,c,2or+dr,2oc+dc]
                             + bias[or,oc,f] )
with B=32, C=32, H=W=128, F=64, OR=OC=64.

Strategy (v3, bf16 + pair-packed matmuls):
  * Spatial-shard over OR (output rows) across 8 cores: 8 or-rows each, no halo.
  * Host-side repack (free): fold (c,dr,dc) into a single K=128 contraction axis on
    the SBUF partition dim; cast x/W to bf16 (tolerance 2e-2 >> bf16 error ~1e-3).
  * ONE matmul per parity PAIR of output pixels: stationary [128, 128] holds both
    pixels' weights (cols par*64+f), moving [128, 64] holds both pixels' x.
    out[par*64+f, xpar*32+b]: diagonal quadrants (par==xpar) are the real results,
    off-diagonal are discarded cross terms. Halves PE instruction count (the
    per-instruction latency, not streaming, dominated v2's PE time).
  * Bias: ONE K=8 "indicator" matmul per PSUM bank (8 pairs) seeds every cell with
    the per-(parity,f,pair) bias; pixel matmuls accumulate onto it (start=False).
  * Epilogue: per bank, relu the two valid quadrants [64, 8, 32] -> compact bf16
    out tile, alternating DVE / Act engines.
  * Reads split across both HWDGE queues: W (8 MiB/core, 1 MiB/row DMAs) on the
    SP(sync) queue; x (4 MiB/core) + bf16 out (2 MiB/core) on the Act queue.
  * Output un-permuted/upcast on the host (free).
"""

import os

import numpy as np
import ml_dtypes

import concourse.bass as bass
import concourse.tile as tile
from concourse import bacc, mybir
from concourse.bass_utils import run_bass_kernel_spmd

F32 = mybir.dt.float32
BF16 = mybir.dt.bfloat16
NP_BF16 = ml_dtypes.bfloat16

B, C, H, W_ = 32, 32, 128, 128
F = 64
OR, OC = 64, 64          # full output spatial dims (stride-2, kernel-2)
NCORES = 8
ORS = OR // NCORES       # or-rows per core = 8
PC = OC // 2             # parity pairs per or-row = 32
GP = 8                   # pairs per PSUM bank: 8*2*32 fp32 = 2 KiB/partition
NG = PC // GP            # bank-groups per or-row = 4

LAST_RESULTS = None      # test harness peeks at this for exec_time_ns


def _build_program():
    nc = bacc.Bacc("TRN2", target_bir_lowering=False)
    xk = nc.dram_tensor("xk", [128, ORS, OC, B], BF16, kind="ExternalInput")
    wk = nc.dram_tensor("wk", [128, ORS, OC, F], BF16, kind="ExternalInput")
    bT = nc.dram_tensor("bT", [GP, ORS, NG, 128], BF16, kind="ExternalInput")
    ind = nc.dram_tensor("ind", [GP, GP * 2 * B], BF16, kind="ExternalInput")
    out = nc.dram_tensor("out", [128, ORS, PC, B], BF16, kind="ExternalOutput")

    with tile.TileContext(nc) as tc:
        with (
            tc.tile_pool(name="wp", bufs=4) as wp,
            tc.tile_pool(name="xp", bufs=3) as xp,
            tc.tile_pool(name="cp", bufs=1) as cp,
            tc.tile_pool(name="op", bufs=3) as op_,
            tc.tile_pool(name="ps", bufs=8, space=bass.MemorySpace.PSUM) as pp,
        ):
            btall = cp.tile([GP, ORS, NG, 128], BF16)
            indt = cp.tile([GP, GP * 2 * B], BF16)
            nc.scalar.dma_start(out=btall[:], in_=bT[:])
            nc.scalar.dma_start(out=indt[:], in_=ind[:])

            xts = []
            for r in range(min(2, ORS)):
                xt = xp.tile([128, OC, B], BF16)
                nc.scalar.dma_start(out=xt[:], in_=xk[:, r])
                xts.append(xt)

            relu_engs = [nc.vector, nc.scalar]
            for r in range(ORS):
                if r + 2 < ORS:
                    xt = xp.tile([128, OC, B], BF16)
                    nc.scalar.dma_start(out=xt[:], in_=xk[:, r + 2])
                    xts.append(xt)
                xt = xts[r]
                wt = wp.tile([128, OC, F], BF16)
                nc.sync.dma_start(out=wt[:], in_=wk[:, r])
                ot = op_.tile([128, PC, B], BF16)
                for g in range(NG):
                    ps = pp.tile([128, GP, 2, B], F32)
                    # Seed the whole bank with bias: psum[p, j, *, *] = biasT[j, p]
                    nc.tensor.matmul(
                        ps[:],
                        btall[:, r, g],            # lhsT [8, 128]
                        indt[:],                   # rhs  [8, 512]
                        start=True,
                        stop=False,
                        skip_group_check=True,
                    )
                    for j in range(GP):
                        oc0 = (g * GP + j) * 2
                        nc.tensor.matmul(
                            ps[:, j],                  # [128, 2, 32]
                            wt[:, oc0 : oc0 + 2, :],   # lhsT [K=128, M=128(par,f)]
                            xt[:, oc0 : oc0 + 2, :],   # rhs  [K=128, N=64(xpar,b)]
                            start=False,
                            stop=j == GP - 1,
                            skip_group_check=True,
                        )
                    # relu only the valid diagonal quadrants into the compact
                    # out tile; off-diagonal cross terms are never read.
                    sl = slice(g * GP, (g + 1) * GP)
                    eng = relu_engs[g % 2]
                    if eng is nc.scalar:
                        eng.activation(
                            ot[0:64, sl, :], ps[0:64, :, 0, :],
                            mybir.ActivationFunctionType.Relu,
                        )
                        eng.activation(
                            ot[64:128, sl, :], ps[64:128, :, 1, :],
                            mybir.ActivationFunctionType.Relu,
                        )
                    else:
                        eng.tensor_scalar_max(ot[0:64, sl, :], ps[0:64, :, 0, :], 0.0)
                        eng.tensor_scalar_max(ot[64:128, sl, :], ps[64:128, :, 1, :], 0.0)
                nc.scalar.dma_start(out=out[:, r], in_=ot[:])
    nc.compile()
    return nc


_NC_CACHE = None


def kernel(x: np.ndarray, W: np.ndarray, b: np.ndarray) -> np.ndarray:
    global LAST_RESULTS, _NC_CACHE
    x = np.ascontiguousarray(x, dtype=np.float32)
    W = np.ascontiguousarray(W, dtype=np.float32)
    b = np.ascontiguousarray(b, dtype=np.float32)

    # ---- host-side repack (k = c*4 + dr*2 + dc on the partition axis) ----
    # xk_full[k, or, oc, b] = x[b, c, 2*or+dr, 2*oc+dc]
    xk_full = np.ascontiguousarray(
        x.reshape(B, C, OR, 2, OC, 2).transpose(1, 3, 5, 2, 4, 0).reshape(128, OR, OC, B)
    ).astype(NP_BF16)
    # wk_full[k, or, oc, f] = W[f, c, 2*or+dr, 2*oc+dc]
    wk_full = np.ascontiguousarray(
        W.reshape(F, C, OR, 2, OC, 2).transpose(1, 3, 5, 2, 4, 0).reshape(128, OR, OC, F)
    ).astype(NP_BF16)
    # reference does a RAW reshape of b (OR,OC,F)->(1,F,OR,OC): bias for output
    # (f,or,oc) is b_raw[f,or,oc] with oc = (g*8+j)*2 + parity.
    # bT_full[j, or, g, parity*64+f] = b_raw[f, or, (g*8+j)*2+parity]
    bT_full = np.ascontiguousarray(
        b.reshape(F, OR, NG, GP, 2).transpose(3, 1, 2, 4, 0).reshape(GP, OR, NG, 128)
    ).astype(NP_BF16)
    # indicator[j, n] = 1 iff n // 64 == j  (bias-broadcast matmul rhs)
    ind_full = np.kron(np.eye(GP, dtype=np.float32), np.ones(2 * B, np.float32)).astype(
        NP_BF16
    )

    if _NC_CACHE is None:
        _NC_CACHE = _build_program()
    nc = _NC_CACHE

    in_maps = []
    for i in range(NCORES):
        sl = slice(i * ORS, (i + 1) * ORS)
        in_maps.append(
            {
                "xk": np.ascontiguousarray(xk_full[:, sl]),
                "wk": np.ascontiguousarray(wk_full[:, sl]),
                "bT": np.ascontiguousarray(bT_full[:, sl]),
                "ind": ind_full,
            }
        )

    trace = bool(os.environ.get("KERNEL_TRACE"))
    res = run_bass_kernel_spmd(nc, in_maps, core_ids=list(range(NCORES)), trace=trace)
    LAST_RESULTS = res

    # ---- host-side unpack ----
    out = np.empty((B, F, OR, OC), dtype=np.float32)
    for i in range(NCORES):
        r = res.results[i]["out"]  # [128=(parity,f), ORS, PC, B] bf16
        blk = (
            r.astype(np.float32)
            .reshape(2, F, ORS, PC, B)
            .transpose(4, 1, 2, 3, 0)  # -> (B, F, ORS, PC, parity)
            .reshape(B, F, ORS, OC)
        )
        out[:, :, i * ORS : (i + 1) * ORS, :] = blk
    return out


# revision 7
# speedup vs baseline: 1.7135x; 1.0138x over previous
"""LocallyConnected2D (per-pixel weights, 2x2 non-overlapping patch sum, bias, relu)
for Trainium2, SPMD over 8 NeuronCores.

Math: out[b,f,or,oc] = relu( sum_{c,dr,dc} x[b,c,2or+dr,2oc+dc] * W[f,c,2or+dr,2oc+dc]
                             + bias[or,oc,f] )
with B=32, C=32, H=W=128, F=64, OR=OC=64.

Strategy (v3, bf16 + pair-packed matmuls):
  * Spatial-shard over OR (output rows) across 8 cores: 8 or-rows each, no halo.
  * Host-side repack (free): fold (c,dr,dc) into a single K=128 contraction axis on
    the SBUF partition dim; cast x/W to bf16 (tolerance 2e-2 >> bf16 error ~1e-3).
  * ONE matmul per parity PAIR of output pixels: stationary [128, 128] holds both
    pixels' weights (cols par*64+f), moving [128, 64] holds both pixels' x.
    out[par*64+f, xpar*32+b]: diagonal quadrants (par==xpar) are the real results,
    off-diagonal are discarded cross terms. Halves PE instruction count (the
    per-instruction latency, not streaming, dominated v2's PE time).
  * Bias: ONE K=8 "indicator" matmul per PSUM bank (8 pairs) seeds every cell with
    the per-(parity,f,pair) bias; pixel matmuls accumulate onto it (start=False).
  * Epilogue: per bank, relu the two valid quadrants [64, 8, 32] -> compact bf16
    out tile, alternating DVE / Act engines.
  * Reads split across both HWDGE queues: W (8 MiB/core, 1 MiB/row DMAs) on the
    SP(sync) queue; x (4 MiB/core) + bf16 out (2 MiB/core) on the Act queue.
  * Output un-permuted/upcast on the host (free).
"""

import os

import numpy as np
import ml_dtypes

import concourse.bass as bass
import concourse.tile as tile
from concourse import bacc, mybir
from concourse.bass_utils import run_bass_kernel_spmd

F32 = mybir.dt.float32
BF16 = mybir.dt.bfloat16
NP_BF16 = ml_dtypes.bfloat16

B, C, H, W_ = 32, 32, 128, 128
F = 64
OR, OC = 64, 64          # full output spatial dims (stride-2, kernel-2)
NCORES = 8
ORS = OR // NCORES       # or-rows per core = 8
PC = OC // 2             # parity pairs per or-row = 32
GP = 8                   # pairs per PSUM bank: 8*2*32 fp32 = 2 KiB/partition
NG = PC // GP            # bank-groups per or-row = 4

LAST_RESULTS = None      # test harness peeks at this for exec_time_ns


def _build_program():
    nc = bacc.Bacc("TRN2", target_bir_lowering=False)
    xk = nc.dram_tensor("xk", [128, ORS, OC, B], BF16, kind="ExternalInput")
    wk = nc.dram_tensor("wk", [128, ORS, OC, F], BF16, kind="ExternalInput")
    bT = nc.dram_tensor("bT", [GP, ORS, NG, 128], BF16, kind="ExternalInput")
    ind = nc.dram_tensor("ind", [GP, GP * 2 * B], BF16, kind="ExternalInput")
    out = nc.dram_tensor("out", [128, ORS, PC, B], BF16, kind="ExternalOutput")

    with tile.TileContext(nc) as tc:
        with (
            tc.tile_pool(name="wp", bufs=ORS) as wp,
            tc.tile_pool(name="xp", bufs=ORS) as xp,
            tc.tile_pool(name="cp", bufs=1) as cp,
            tc.tile_pool(name="op", bufs=4) as op_,
            tc.tile_pool(name="ps", bufs=8, space=bass.MemorySpace.PSUM) as pp,
        ):
            # Constants first: they gate the first bias matmul.
            btall = cp.tile([GP, ORS, NG, 128], BF16)
            indt = cp.tile([GP, GP * 2 * B], BF16)
            nc.scalar.dma_start(out=btall[:], in_=bT[:])
            nc.scalar.dma_start(out=indt[:], in_=ind[:])

            # Front-load every input DMA: W streams on the SP(sync) HWDGE
            # queue, x on the Act(scalar) queue. Row 0 is split into chunks
            # so the PE can start ~4us earlier than a monolithic 1 MiB DMA.
            wts, xts = [], []
            for r in range(ORS):
                wt = wp.tile([128, OC, F], BF16)
                if r == 0:
                    for c in range(4):
                        sl = slice(16 * c, 16 * (c + 1))
                        nc.sync.dma_start(out=wt[:, sl, :], in_=wk[:, r, sl])
                else:
                    nc.sync.dma_start(out=wt[:], in_=wk[:, r])
                wts.append(wt)
            for r in range(ORS):
                xt = xp.tile([128, OC, B], BF16)
                if r == 0:
                    for c in range(2):
                        sl = slice(32 * c, 32 * (c + 1))
                        nc.scalar.dma_start(out=xt[:, sl, :], in_=xk[:, r, sl])
                else:
                    nc.scalar.dma_start(out=xt[:], in_=xk[:, r])
                xts.append(xt)

            for r in range(ORS):
                xt = xts[r]
                wt = wts[r]
                ot = op_.tile([128, PC, B], BF16)
                for g in range(NG):
                    ps = pp.tile([128, GP, 2, B], F32)
                    # Seed the whole bank with bias: psum[p, j, *, *] = biasT[j, p]
                    nc.tensor.matmul(
                        ps[:],
                        btall[:, r, g],            # lhsT [8, 128]
                        indt[:],                   # rhs  [8, 512]
                        start=True,
                        stop=False,
                        skip_group_check=True,
                    )
                    for j in range(GP):
                        oc0 = (g * GP + j) * 2
                        nc.tensor.matmul(
                            ps[:, j],                  # [128, 2, 32]
                            wt[:, oc0 : oc0 + 2, :],   # lhsT [K=128, M=128(par,f)]
                            xt[:, oc0 : oc0 + 2, :],   # rhs  [K=128, N=64(xpar,b)]
                            start=False,
                            stop=j == GP - 1,
                            skip_group_check=True,
                        )
                    # relu only the valid diagonal quadrants into the compact
                    # out tile; off-diagonal cross terms are never read. All
                    # relus live on DVE so the Act engine only issues DMAs.
                    sl = slice(g * GP, (g + 1) * GP)
                    nc.vector.tensor_scalar_max(ot[0:64, sl, :], ps[0:64, :, 0, :], 0.0)
                    nc.vector.tensor_scalar_max(ot[64:128, sl, :], ps[64:128, :, 1, :], 0.0)
                nc.scalar.dma_start(out=out[:, r], in_=ot[:])
    nc.compile()
    return nc


_NC_CACHE = None


def kernel(x: np.ndarray, W: np.ndarray, b: np.ndarray) -> np.ndarray:
    global LAST_RESULTS, _NC_CACHE
    x = np.ascontiguousarray(x, dtype=np.float32)
    W = np.ascontiguousarray(W, dtype=np.float32)
    b = np.ascontiguousarray(b, dtype=np.float32)

    # ---- host-side repack (k = c*4 + dr*2 + dc on the partition axis) ----
    # xk_full[k, or, oc, b] = x[b, c, 2*or+dr, 2*oc+dc]
    xk_full = np.ascontiguousarray(
        x.reshape(B, C, OR, 2, OC, 2).transpose(1, 3, 5, 2, 4, 0).reshape(128, OR, OC, B)
    ).astype(NP_BF16)
    # wk_full[k, or, oc, f] = W[f, c, 2*or+dr, 2*oc+dc]
    wk_full = np.ascontiguousarray(
        W.reshape(F, C, OR, 2, OC, 2).transpose(1, 3, 5, 2, 4, 0).reshape(128, OR, OC, F)
    ).astype(NP_BF16)
    # reference does a RAW reshape of b (OR,OC,F)->(1,F,OR,OC): bias for output
    # (f,or,oc) is b_raw[f,or,oc] with oc = (g*8+j)*2 + parity.
    # bT_full[j, or, g, parity*64+f] = b_raw[f, or, (g*8+j)*2+parity]
    bT_full = np.ascontiguousarray(
        b.reshape(F, OR, NG, GP, 2).transpose(3, 1, 2, 4, 0).reshape(GP, OR, NG, 128)
    ).astype(NP_BF16)
    # indicator[j, n] = 1 iff n // 64 == j  (bias-broadcast matmul rhs)
    ind_full = np.kron(np.eye(GP, dtype=np.float32), np.ones(2 * B, np.float32)).astype(
        NP_BF16
    )

    if _NC_CACHE is None:
        _NC_CACHE = _build_program()
    nc = _NC_CACHE

    in_maps = []
    for i in range(NCORES):
        sl = slice(i * ORS, (i + 1) * ORS)
        in_maps.append(
            {
                "xk": np.ascontiguousarray(xk_full[:, sl]),
                "wk": np.ascontiguousarray(wk_full[:, sl]),
                "bT": np.ascontiguousarray(bT_full[:, sl]),
                "ind": ind_full,
            }
        )

    trace = bool(os.environ.get("KERNEL_TRACE"))
    res = run_bass_kernel_spmd(nc, in_maps, core_ids=list(range(NCORES)), trace=trace)
    LAST_RESULTS = res

    # ---- host-side unpack ----
    out = np.empty((B, F, OR, OC), dtype=np.float32)
    for i in range(NCORES):
        r = res.results[i]["out"]  # [128=(parity,f), ORS, PC, B] bf16
        blk = (
            r.astype(np.float32)
            .reshape(2, F, ORS, PC, B)
            .transpose(4, 1, 2, 3, 0)  # -> (B, F, ORS, PC, parity)
            .reshape(B, F, ORS, OC)
        )
        out[:, :, i * ORS : (i + 1) * ORS, :] = blk
    return out
